# revision 29
# baseline (speedup 1.0000x reference)
"""Trainium2 Bass kernel for nn_CHPS_model_20976620273883 (retrieval_knn).

Computes, for x[8192,4096] f32, W[4096,1024] f32, b[1024] f32,
prototypes[1000,1024] f32:

    emb   = x @ W + b
    cos   = normalize(emb) @ normalize(prototypes).T
    out   = (cos - 1) / 0.01            # == 100*cos - 100

Sharding: data-parallel on the batch — each of the 8 NeuronCores gets
1024 rows of x; W / b / prototypes are replicated.  No collectives.

Device algorithm (per core), all matmuls in bf16 with fp32 PSUM accum:
  phase 1: embT[D,Bl] = W.T @ x.T    (W k-slices stationary, xT moving;
           xT produced by 2-byte xbar DMA-transpose straight from DRAM)
  norms:   q[b] = sum_d (embT[d,b]+bias)^2 via ACT Square + DVE adds,
           PE-transpose of the partial sums + DVE row-reduce, then
           s100[b] = 1/sqrt(q*1e-4) = 100/||emb_b||  (ACT Sqrt + DVE recip)
  phase 2: raw[Bl,P] = embT.T @ protoT_n  (embT slices stationary,
           prototypes normalized on-chip, transposed via 2-byte xbar DMA)
  epilogue: out = raw*s100[b] - 100     (one DVE tensor_scalar from PSUM)
"""

import numpy as np
import ml_dtypes

B, F_IN, D, P = 8192, 4096, 1024, 1000
NCORES = 8
BL = B // NCORES          # 1024 rows per core
KT = F_IN // 128          # 32 contraction tiles
DT = D // 128             # 8 embedding-dim tiles
NB = 512                  # phase-1 moving width (one fp32 PSUM bank)
NCH = BL // NB            # 2 batch chunks per core
PT = 128                  # proto rows per natural tile
P_PAD = 1024              # prototypes padded to 8 tiles of 128

_cache = {}


def _emit(nc, tc, mybir, x_d, w_d, b_d, p_d, o_d, id_f32):
    f32 = mybir.dt.float32
    bf16 = mybir.dt.bfloat16
    AF = mybir.ActivationFunctionType
    Alu = mybir.AluOpType

    with (
        tc.tile_pool(name="const", bufs=1) as constp,
        tc.tile_pool(name="wpool", bufs=1) as wpool,
        tc.tile_pool(name="xpool", bufs=1) as xpool,
        tc.tile_pool(name="embp", bufs=1) as embp,
        tc.tile_pool(name="ptp", bufs=1) as ptp,
        tc.tile_pool(name="pnat", bufs=2) as pnat,
        tc.tile_pool(name="work", bufs=3) as work,
        tc.tile_pool(name="sml", bufs=2) as sml,
        tc.tile_pool(name="outp", bufs=4) as outp,
        tc.tile_pool(name="ps1", bufs=4, space="PSUM") as ps1p,
        tc.tile_pool(name="ps2", bufs=2, space="PSUM") as ps2p,
        tc.tile_pool(name="pst", bufs=2, space="PSUM") as pstp,
    ):
        # ---- constants -------------------------------------------------
        idf = constp.tile([128, 128], f32)
        nc.sync.dma_start(idf[:], id_f32.ap())
        # b rearranged to column layout: bcol[p, d] = b[d*128 + p]
        bcol = constp.tile([128, DT], f32)
        nc.sync.dma_start(bcol[:], b_d.ap().rearrange("(d p) -> p d", p=128))
        # tiny epsilon bias tile (emulates reference's max(norm, eps) clamp
        # and keeps the zero-padded prototype rows NaN-free)
        epsb = constp.tile([128, 1], f32)
        nc.vector.memset(epsb[:], 1e-24)

        # ---- weight / xT loads. Tile serializes xbar-mode transitions
        # (copy DMA <-> transpose DMA, ~19us drain each in the cost model),
        # so keep one clean transition: all W copies, then all transposes.
        wt = []
        for k in range(KT):
            wtk = wpool.tile([128, D], bf16, name=f"w{k}")
            nc.sync.dma_start(wtk[:], w_d.ap()[k * 128:(k + 1) * 128, :])
            wt.append(wtk)
        xt = []
        for k in range(KT):
            xtk = xpool.tile([128, BL], bf16, name=f"xT{k}")
            nc.sync.dma_start(
                xtk[:], x_d.ap()[:, k * 128:(k + 1) * 128], transpose=True
            )
            xt.append(xtk)

        # persistent bf16 embT tiles: embT[t] rows = emb dims t*128..t*128+127
        embt = [embp.tile([128, BL], bf16, name=f"embT{t}") for t in range(DT)]
        # transposed normalized prototypes: ptt[t][:, p] = proto_n[p, t*128+..]
        ptt = [ptp.tile([128, P_PAD], bf16, name=f"ptT{t}") for t in range(DT)]

        # per-chunk 100/||emb_b|| columns  (s100[cc][:, j] for b-tile cc*4+j)
        s100 = [sml.tile([128, 4], f32, name=f"s100_{cc}") for cc in range(NCH)]

        # ================= phase 1: embT = W.T @ xT =====================
        def phase1_chunk(cc):
            bs = cc * NB
            partial = work.tile([128, NB], f32, name=f"psum_sq{cc}", tag="partial")
            for d in range(DT):
                ps = ps1p.tile([128, NB], f32, name="ps1")
                for k in range(KT):
                    nc.tensor.matmul(
                        ps[:],
                        wt[k][:, d * 128:(d + 1) * 128],
                        xt[k][:, bs:bs + NB],
                        start=(k == 0),
                        stop=(k == KT - 1),
                    )
                # emb (bias added) -> bf16 for phase 2
                nc.vector.tensor_scalar(
                    embt[d][:, bs:bs + NB], ps[:], bcol[:, d:d + 1], None, Alu.add
                )
                # squared emb (bias folded into ACT) -> f32
                sq = work.tile([128, NB], f32, name="sq", tag="sq")
                nc.scalar.activation(
                    sq[:], ps[:], AF.Square, bias=bcol[:, d:d + 1], scale=1.0
                )
                if d == 0:
                    nc.vector.tensor_copy(partial[:], sq[:])
                else:
                    nc.vector.tensor_add(partial[:], partial[:], sq[:])
            # norms: transpose partial 128x128 blocks, reduce rows
            qcol = sml.tile([128, 4], f32, name=f"qcol{cc}", tag="qcol")
            for j in range(4):
                pt = pstp.tile([128, 128], f32, name="pst", tag="tp")
                nc.tensor.transpose(pt[:], partial[:, j * 128:(j + 1) * 128], idf[:])
                nc.vector.tensor_reduce(
                    qcol[:, j:j + 1], pt[:], mybir.AxisListType.X, Alu.add
                )
            # s100 = 1/sqrt(q*1e-4 + eps) = 100/||emb||   (clamp-safe)
            rt = sml.tile([128, 4], f32, name=f"rt{cc}", tag="rt")
            nc.scalar.activation(rt[:], qcol[:], AF.Sqrt, bias=epsb[:], scale=1e-4)
            nc.vector.reciprocal(s100[cc][:], rt[:])

        # ================= phase 0b: prototypes =========================
        # All copy-DMAs + normalization first, then every transpose-DMA in
        # one block: xbar-mode transitions (copy<->transpose) serialize the
        # DMA stream, so keep them to a minimum.
        def proto_prep():
            pnns = []
            for t in range(DT):
                pn = pnat.tile([128, D], bf16, name=f"pn{t}", tag="pn")
                rows = min(PT, P - t * PT)
                if rows < PT:
                    nc.vector.memset(pn[:], 0.0)
                nc.sync.dma_start(
                    pn[:rows, :], p_d.ap()[t * PT:t * PT + rows, :]
                )
                psq = work.tile([128, D], f32, name="psq", tag="psq", bufs=1)
                nc.scalar.activation(psq[:], pn[:], AF.Square)
                pq = sml.tile([128, 1], f32, name="pq", tag="pq")
                nc.vector.tensor_reduce(
                    pq[:], psq[:], mybir.AxisListType.X, Alu.add
                )
                pr = sml.tile([128, 1], f32, name="pr", tag="pq")
                nc.scalar.activation(pr[:], pq[:], AF.Sqrt, bias=epsb[:])
                pri = sml.tile([128, 1], f32, name="pri", tag="pq")
                nc.vector.reciprocal(pri[:], pr[:])
                pnn = pnat.tile([128, D], bf16, name=f"pnn{t}", tag="pnn", bufs=DT)
                nc.vector.tensor_scalar(pnn[:], pn[:], pri[:], None, Alu.mult)
                pnns.append(pnn)
            for t in range(DT):
                for c in range(DT):
                    # 2-byte xbar SBUF->SBUF transpose keeps this off the PE
                    nc.sync.dma_start(
                        ptt[c][:, t * 128:(t + 1) * 128],
                        pnns[t][:, c * 128:(c + 1) * 128],
                        transpose=True,
                    )

        # ================= phase 2: out = embT.T @ protoT ===============
        def phase2_chunk(cc):
            for j in range(4):
                bt = cc * 4 + j
                for pc, (pn0, pnn_) in enumerate([(0, NB), (NB, P - NB)]):
                    ps2 = ps2p.tile([128, NB], f32, name="ps2")
                    for t in range(DT):
                        nc.tensor.matmul(
                            ps2[:, :pnn_],
                            embt[t][:, bt * 128:(bt + 1) * 128],
                            ptt[t][:, pn0:pn0 + pnn_],
                            start=(t == 0),
                            stop=(t == DT - 1),
                        )
                    ot = outp.tile([128, NB], f32, name="ot")
                    nc.vector.tensor_scalar(
                        ot[:, :pnn_], ps2[:, :pnn_], s100[cc][:, j:j + 1],
                        -100.0, Alu.mult, Alu.add,
                    )
                    nc.sync.dma_start(
                        o_d.ap()[bt * 128:(bt + 1) * 128, pn0:pn0 + pnn_],
                        ot[:, :pnn_],
                    )

        # emission order: big chunk-0 matmul first so the proto pipeline
        # (DMA/ACT/DVE) and its PE transposes hide under it.
        phase1_chunk(0)
        proto_prep()
        phase2_chunk(0)
        phase1_chunk(1)
        phase2_chunk(1)


def _build(reps=1):
    key = ("mod", reps)
    if key in _cache:
        return _cache[key]
    import concourse.bacc as bacc
    import concourse.mybir as mybir
    import concourse.tile as tile

    nc = bacc.Bacc(
        "TRN2", target_bir_lowering=False, debug=False, num_devices=NCORES
    )
    f32 = mybir.dt.float32
    bf16 = mybir.dt.bfloat16
    x_d = nc.dram_tensor("x", [BL, F_IN], bf16, kind="ExternalInput")
    w_d = nc.dram_tensor("w", [F_IN, D], bf16, kind="ExternalInput")
    b_d = nc.dram_tensor("b", [D], f32, kind="ExternalInput")
    p_d = nc.dram_tensor("protos", [P, D], bf16, kind="ExternalInput")
    o_d = nc.dram_tensor("out", [BL, P], f32, kind="ExternalOutput")
    id_f32 = nc.inline_tensor(np.eye(128, dtype=np.float32), name="id_f32")

    with tile.TileContext(nc) as tc:
        for _ in range(reps):
            _emit(nc, tc, mybir, x_d, w_d, b_d, p_d, o_d, id_f32)
    nc.compile()
    _cache[key] = nc
    return nc


def _in_maps(inputs):
    x = np.ascontiguousarray(inputs["x"]).astype(ml_dtypes.bfloat16)
    w = np.ascontiguousarray(inputs["W"]).astype(ml_dtypes.bfloat16)
    bb = np.ascontiguousarray(inputs["b"]).astype(np.float32)
    pp = np.ascontiguousarray(inputs["prototypes"]).astype(ml_dtypes.bfloat16)
    return [
        {"x": x[c * BL:(c + 1) * BL], "w": w, "b": bb, "protos": pp}
        for c in range(NCORES)
    ]


def kernel(**inputs) -> np.ndarray:
    from concourse import bass_utils

    nc = _build(reps=1)
    in_maps = _in_maps(inputs)
    try:
        res = bass_utils.run_bass_kernel_spmd(
            nc, in_maps, core_ids=list(range(NCORES))
        )
    except Exception:
        # transient axon-session hiccups are recoverable on a second attempt
        res = bass_utils.run_bass_kernel_spmd(
            nc, in_maps, core_ids=list(range(NCORES))
        )
    return np.concatenate([res.results[c]["out"] for c in range(NCORES)], axis=0)
